# revision 19
# baseline (speedup 1.0000x reference)
"""Trainium2 Bass kernel for nn_DigitConvolutionalModel.

Model: out = relu(conv2d_valid(x.reshape(28,28), conv_w).reshape(676) @ w1 + b1) @ w2 + b2

Strategy:
  - The 3x3 valid conv is a linear map C [784, 676]; fold it into the first
    FC layer on the host: W1' = C @ w1  [784, 300]. The device then runs a
    plain 2-layer MLP: out = relu(x @ W1' + b1) @ w2 + b2.
  - Pure data parallel over 8 NeuronCores: batch 65536 -> 8192 per core.
  - Feature-major device layout: host supplies x.T per core so the
    contraction dim sits on SBUF partitions for both matmul operands.
    Layer 1 computes g = relu(W1'.T @ x.T + b1) as [300, batch]; layer 2
    reuses g directly as the moving operand: out.T = w2.T @ g + b2.
  - All feature dims zero-padded to multiples of 128 (784->896, 300->384);
    the zero-padding contributes exactly 0 through matmul/relu.
  - bf16 matmul inputs (1 PE cycle/row vs 4 for fp32), fp32 PSUM accumulate.
  - Dependency tracking is tile-granular: a matmul reading one k-chunk slice
    of a tile waits for ALL writers of that tile. Pair-0 x and w1 therefore
    live in per-chunk tiles with per-chunk DMAs (k0 issued first) so the
    first real matmul starts as soon as chunk 0 lands (~8us) instead of
    after the whole prologue transfer (~13us).
  - m2 (44-row tail of the 300 hidden features) and layer 2 (M=10) both run
    col-tiled: tile_position col groups 0/64 give two concurrent streams on
    disjoint PE column groups, so each ki/mi slot costs one 512-col stream
    for both batch halves.
  - Per-pair block order [m01 | m2 | l2(prev)] keeps full-array/col-tiled
    mode switches to 3 per pair (each costs ~105ns of unhidden LDWEIGHTS).
  - Layer-2 output drains as ONE ACT pass over the whole [128,512] ps2 bank
    (j0 rows 0-9, j1 rows 64-73, bias from a [128,1] b2 replica); the two
    outT stores issue on gpsimd/vector so the final pair's stores overlap.
"""

import numpy as np
import ml_dtypes

_B = 65536
_NCORES = 8
_BSH = _B // _NCORES  # 8192 batch rows per core
_N = 512  # batch columns per matmul (one fp32 PSUM bank)
_KP = 896  # padded input features (784 -> 7 chunks of 128)
_MP = 384  # padded hidden features (300 -> 3 chunks of 128)
_NK = _KP // 128  # 7
_NM = _MP // 128  # 3
_NPAIR = _BSH // (2 * _N)  # 8 pairs of 512-col batch tiles
_M2 = 300 - 256  # 44 real rows in the third m-chunk

_state = {}


def _build_nc():
    import concourse.tile as tile
    from concourse import bacc, mybir
    from contextlib import ExitStack

    dt = mybir.dt
    AF = mybir.ActivationFunctionType

    nc = bacc.Bacc(
        "TRN2",
        target_bir_lowering=False,
        debug=False,
        enable_asserts=False,
        num_devices=_NCORES,
    )

    xt = nc.dram_tensor("xt", [_KP, _BSH], dt.bfloat16, kind="ExternalInput").ap()
    w1 = nc.dram_tensor("w1", [_KP, _MP], dt.bfloat16, kind="ExternalInput").ap()
    b1 = nc.dram_tensor("b1", [_MP, 1], dt.float32, kind="ExternalInput").ap()
    w2 = nc.dram_tensor("w2", [_MP, 10], dt.bfloat16, kind="ExternalInput").ap()
    b2f = nc.dram_tensor("b2f", [128, 1], dt.float32, kind="ExternalInput").ap()
    w2rep = nc.dram_tensor("w2rep", [128, 10], dt.bfloat16, kind="ExternalInput").ap()
    b1rep = nc.dram_tensor("b1rep", [128, 1], dt.float32, kind="ExternalInput").ap()
    outT = nc.dram_tensor("outT", [10, _BSH], dt.float32, kind="ExternalOutput").ap()

    # Partition-chunked DRAM views: [(chunk, p), cols] -> [p, chunk, cols]
    xt_r = xt.rearrange("(k p) c -> p k c", p=128)  # [128, 7, 8192]
    w1_r = w1.rearrange("(k p) m -> p k m", p=128)  # [128, 7, 384]
    b1_r = b1.rearrange("(m p) one -> p m one", p=128)  # [128, 3, 1]
    w2_r = w2.rearrange("(m p) o -> p m o", p=128)  # [128, 3, 10]

    with tile.TileContext(nc) as tc, ExitStack() as ctx:
        wpool = ctx.enter_context(tc.tile_pool(name="wpool", bufs=1))
        xp0 = ctx.enter_context(tc.tile_pool(name="xp0", bufs=1))
        xpool = ctx.enter_context(tc.tile_pool(name="xpool", bufs=2))
        gpool = ctx.enter_context(tc.tile_pool(name="gpool", bufs=2))
        ppool = ctx.enter_context(tc.tile_pool(name="ppool", bufs=5, space="PSUM"))
        pm2pool = ctx.enter_context(tc.tile_pool(name="pm2pool", bufs=1, space="PSUM"))
        p2pool = ctx.enter_context(tc.tile_pool(name="p2pool", bufs=2, space="PSUM"))
        opool = ctx.enter_context(tc.tile_pool(name="opool", bufs=2))

        # PE warm-up: a few dependency-free matmuls on a zeroed scratch tile
        # bridge the gap between engine start (~7.4us) and the first x/w
        # chunks landing (~8.3us) so the HAM activity window starts filling
        # and the PE never idles before the real stream begins. The memset
        # runs on vector so gpsimd's queue starts with the x-chunk DMAs.
        warm_in = wpool.tile([128, 128], dt.bfloat16, name="warm_in", tag="warm_in")
        nc.vector.memset(warm_in[:], 0.0)
        warm_ps = p2pool.tile([128, 128], dt.float32, name="warm_ps", tag="ps2")
        for _ in range(8):
            nc.tensor.matmul(
                out=warm_ps[:], lhsT=warm_in[:], rhs=warm_in[:], start=True, stop=True
            )

        # Pair-0 x and the stationary weights in PER-CHUNK tiles so the first
        # matmul only waits on its own (w1[k0], xt[k0]) chunk DMAs. Chunk
        # DMAs are ordered k0-first on two otherwise-idle engine queues
        # (weights on sync, x on gpsimd); each dma_start costs ~0.65us of
        # descriptor generation on the issuing engine.
        # x0 chunks go on scalar and w1 chunks on sync: both are HARDWARE
        # DGE rings that start transfers promptly. The gpsimd ring is a
        # software DGE whose transfers lag ~2-3us, so it only gets the
        # late-needed small tensors and the pair-1 prefetch.
        w1c = []
        xt0c = []
        for ki in range(_NK):
            wck = wpool.tile([128, _MP], dt.bfloat16, name=f"w1c_{ki}", tag=f"w1c{ki}")
            nc.sync.dma_start(out=wck[:], in_=w1_r[:, ki, :])
            w1c.append(wck)
            xck = xp0.tile([128, 2 * _N], dt.bfloat16, name=f"xt0_{ki}", tag=f"x0{ki}")
            nc.scalar.dma_start(out=xck[:], in_=xt_r[:, ki, 0 : 2 * _N])
            xt0c.append(xck)
        # Small tensors ride the sync HW ring after the w chunks; b1 first
        # (needed earliest, by pair-0's relu drain).
        b1sb = wpool.tile([128, _NM, 1], dt.float32, name="b1sb", tag="b1sb")
        nc.sync.dma_start(out=b1sb[:], in_=b1_r[:])
        w2sb = wpool.tile([128, _NM, 10], dt.bfloat16, name="w2sb", tag="w2sb")
        nc.sync.dma_start(out=w2sb[:], in_=w2_r[:])
        # b2 replicated at partitions 0-9 (j0) and 64-73 (j1) so one ACT pass
        # with per-partition bias drains both halves of the layer-2 psum.
        b2sb = wpool.tile([128, 1], dt.float32, name="b2sb", tag="b2sb")
        nc.sync.dma_start(out=b2sb[:], in_=b2f[:, :])
        # The m2 chunk (44 real rows of 300) is col-tiled: batch half j=1
        # lands at psum/sbuf partitions 64.. so its bias and layer-2 weights
        # need partition-64-aligned replicas (built fully on host: one DMA).
        b1rsb = wpool.tile([128, 1], dt.float32, name="b1rsb", tag="b1rsb")
        nc.sync.dma_start(out=b1rsb[:], in_=b1rep[:, :])
        w2rsb = wpool.tile([128, 10], dt.bfloat16, name="w2rsb", tag="w2rsb")
        nc.sync.dma_start(out=w2rsb[:], in_=w2rep[:, :])

        _KA = 4  # k-chunks in the first half-tile of a prefetched pair

        def xrhs(pair, xtile, ki, cols, rows=slice(0, 128)):
            if pair == 0:
                return xt0c[ki][rows, cols]
            xa, xb = xtile
            if ki < _KA:
                return xa[rows, ki, cols]
            return xb[rows, ki - _KA, cols]

        def layer2(prev_g, prev_c0):
            """Second layer for the pair at column prev_c0: both batch halves
            run concurrently on PE col groups 0 / 64 into one psum bank."""
            ps2 = p2pool.tile([128, _N], dt.float32, name=f"ps2_{prev_c0}", tag="ps2")
            for mi in range(_NM):
                for j in range(2):
                    if j == 0:
                        lw = w2sb[:, mi, :]
                    else:
                        # j1's m2 g-rows live at partitions 64-107; use the
                        # partition-aligned replica for that chunk.
                        lw = w2rsb[:] if mi == 2 else w2sb[:, mi, :]
                    nc.tensor.matmul(
                        out=ps2[64 * j : 64 * j + 10, :],
                        lhsT=lw,
                        rhs=prev_g[(mi, j)][:],
                        start=(mi == 0),
                        stop=(mi == _NM - 1),
                        tile_position=(0, 64 * j),
                    )
            # One ACT pass drains both halves (rows 0-9 and 64-73); the
            # untouched rows are never stored.
            ob = opool.tile([128, _N], dt.float32, name=f"ob_{prev_c0}", tag="ob")
            nc.scalar.activation(ob[:], ps2[:], AF.Identity, bias=b2sb[:], scale=1.0)
            nc.sync.dma_start(out=outT[:, prev_c0 : prev_c0 + _N], in_=ob[0:10, :])
            nc.scalar.dma_start(
                out=outT[:, prev_c0 + _N : prev_c0 + 2 * _N], in_=ob[64:74, :]
            )

        prev_g = None
        prev_c0 = 0
        for pair in range(_NPAIR):
            c0 = pair * 2 * _N
            xtile = None
            if pair > 0:
                # Prefetched pairs arrive as two half-tiles (k0-3, k4-6) so
                # the pair's first matmuls only wait on the first ~1MB and
                # the second half's DMA deadline is ~3.5us later. The start
                # is HBM-bandwidth-bound, so the issue order must match the
                # consumption order: pair-1 goes on scalar BEHIND the pair-0
                # chunk DMAs (same HW ring serves them first), pairs 2+ on
                # sync behind the w1 chunks and smalls.
                xa = xpool.tile(
                    [128, _KA, 2 * _N], dt.bfloat16, name=f"xta_{pair}", tag="xa"
                )
                xb = xpool.tile(
                    [128, _NK - _KA, 2 * _N], dt.bfloat16, name=f"xtb_{pair}", tag="xb"
                )
                eng = nc.scalar if pair == 1 else nc.sync
                eng.dma_start(out=xa[:], in_=xt_r[:, 0:_KA, c0 : c0 + 2 * _N])
                eng.dma_start(out=xb[:], in_=xt_r[:, _KA:_NK, c0 : c0 + 2 * _N])
                xtile = (xa, xb)

            cur_g = {}
            # k-major sweep over m0/m1: consume each x-chunk for all four
            # (mi, j) accumulators before needing the next chunk, so the
            # DMA-raced first pair doesn't stall the TensorEngine.
            ps = {
                (mi, j): ppool.tile(
                    [128, _N], dt.float32, name=f"ps_{pair}_{mi}_{j}", tag="ps"
                )
                for mi in range(2)
                for j in range(2)
            }
            for ki in range(_NK - 1):
                for mi in range(2):
                    for j in range(2):
                        nc.tensor.matmul(
                            out=ps[(mi, j)][:],
                            lhsT=w1c[ki][:, mi * 128 : (mi + 1) * 128],
                            rhs=xrhs(pair, xtile, ki, slice(j * _N, (j + 1) * _N)),
                            start=(ki == 0),
                            stop=False,
                        )
            # ki=6 has only 16 real contraction rows (784 = 6*128 + 16). The
            # host replicates those rows (x and w1 alike) at partition
            # offsets 32/64/96 of chunk 6, so the four (mi, j) tails run as
            # K=16 row-tiled matmuls on distinct 32-row groups — which the
            # PE executes CONCURRENTLY (one ~512-cycle slot instead of 4).
            for gi, (mi, j) in enumerate([(0, 0), (0, 1), (1, 0), (1, 1)]):
                rg = 32 * gi
                nc.tensor.matmul(
                    out=ps[(mi, j)][:],
                    lhsT=w1c[_NK - 1][rg : rg + 16, mi * 128 : (mi + 1) * 128],
                    rhs=xrhs(
                        pair, xtile, _NK - 1,
                        slice(j * _N, (j + 1) * _N), rows=slice(rg, rg + 16),
                    ),
                    start=False,
                    stop=True,
                    tile_position=(rg, 0),
                )
            for mi in range(2):
                for j in range(2):
                    g = gpool.tile(
                        [128, _N], dt.bfloat16, name=f"g_{pair}_{mi}_{j}", tag=f"g{mi}{j}"
                    )
                    if j == 0:
                        # Split the relus across ACT and DVE so neither engine
                        # serializes the psum drain.
                        nc.scalar.activation(
                            g[:], ps[(mi, j)][:], AF.Relu, bias=b1sb[:, mi, :], scale=1.0
                        )
                    else:
                        nc.vector.tensor_scalar(
                            g[:], ps[(mi, j)][:], b1sb[:, mi, :], 0.0,
                            mybir.AluOpType.add, mybir.AluOpType.max,
                        )
                    cur_g[(mi, j)] = g

            # m2 chunk (44 output rows): both batch halves run concurrently as
            # col-tiled matmuls — j=0 writes psum partitions 0..43 (col group
            # 0), j=1 writes partitions 64..107 (col group 64) of one bank.
            psm2 = pm2pool.tile([128, _N], dt.float32, name=f"psm2_{pair}", tag="psm2")
            for ki in range(_NK):
                # Chunk 6 carries the 16 real rows replicated at partition
                # offsets 32/64/96 (for the m01 row-tiled tails); m2 must
                # contract over partitions 0-15 only or it would count the
                # replicas four times.
                kr = slice(0, 16) if ki == _NK - 1 else slice(0, 128)
                for j in range(2):
                    nc.tensor.matmul(
                        out=psm2[64 * j : 64 * j + _M2, :],
                        lhsT=w1c[ki][kr, 256 : 256 + _M2],
                        rhs=xrhs(pair, xtile, ki, slice(j * _N, (j + 1) * _N), rows=kr),
                        start=(ki == 0),
                        stop=(ki == _NK - 1),
                        tile_position=(0, 64 * j),
                    )
            # g tiles are full 128 rows with the unused rows zeroed so layer 2
            # can use uniform full-row matmuls (0-weight x 0-value, never NaN).
            # The gpool ring has 2 buffers per tag and nothing but these
            # memsets ever writes the padding rows, so zeroing the first two
            # generations keeps every later generation zero too.
            g20 = gpool.tile([128, _N], dt.bfloat16, name=f"g_{pair}_2_0", tag="g20")
            if pair < 2:
                nc.vector.memset(g20[32:64, :], 0.0)  # 32-aligned; relu rewrites 32..43
                nc.vector.memset(g20[64:128, :], 0.0)
            nc.scalar.activation(
                g20[0:_M2, :], psm2[0:_M2, :], AF.Relu, bias=b1sb[0:_M2, 2, :], scale=1.0
            )
            g21 = gpool.tile([128, _N], dt.bfloat16, name=f"g_{pair}_2_1", tag="g21")
            if pair < 2:
                nc.vector.memset(g21[0:64, :], 0.0)
                nc.vector.memset(g21[96:128, :], 0.0)  # 32-aligned; relu rewrites 96..107
            nc.vector.tensor_scalar(
                g21[64 : 64 + _M2, :], psm2[64 : 64 + _M2, :], b1rsb[64 : 64 + _M2, :],
                0.0, mybir.AluOpType.add, mybir.AluOpType.max,
            )
            cur_g[(2, 0)] = g20
            cur_g[(2, 1)] = g21

            # Software-pipelined layer 2 for the previous pair, placed after
            # m2 so the col-tiled blocks are adjacent (fewer PE mode switches).
            if prev_g is not None:
                layer2(prev_g, prev_c0)
            prev_g = cur_g
            prev_c0 = c0
        layer2(prev_g, prev_c0)

    nc.compile()
    return nc


def _fold_conv(conv_w, w1):
    """W1' = C @ w1 where C [784, 676] is the linear map of the 3x3 valid conv."""
    C = np.zeros((784, 676), np.float64)
    cw = np.asarray(conv_w, np.float64)
    for di in range(3):
        for dj in range(3):
            for i in range(26):
                rows = (i + di) * 28 + dj + np.arange(26)
                C[rows, i * 26 + np.arange(26)] += cw[di, dj]
    return C @ np.asarray(w1, np.float64)  # [784, 300]


def _exec(inputs, trace=False, **run_kwargs):
    from concourse.bass_utils import run_bass_kernel_spmd

    x = np.asarray(inputs["x"], np.float32)
    bf16 = ml_dtypes.bfloat16

    w1f = np.zeros((_KP, _MP), bf16)
    w1f[:784, :300] = _fold_conv(inputs["conv_w"], inputs["w1"]).astype(bf16)
    # Replicate the 16 real rows of k-chunk 6 at partition offsets 32/64/96
    # for the row-tiled ki=6 tail matmuls (see _build_nc).
    for g in (1, 2, 3):
        w1f[768 + 32 * g : 784 + 32 * g] = w1f[768:784]
    b1c = np.zeros((_MP, 1), np.float32)
    b1c[:300, 0] = np.asarray(inputs["b1"], np.float32)
    w2b = np.zeros((_MP, 10), bf16)
    w2b[:300] = np.asarray(inputs["w2"], np.float32).astype(bf16)
    b2v = np.asarray(inputs["b2"], np.float32).reshape(10)
    b2f = np.zeros((128, 1), np.float32)
    b2f[0:10, 0] = b2v
    b2f[64:74, 0] = b2v
    # Partition-64-aligned replicas for the m2 chunk's j=1 half.
    w2rep = np.zeros((128, 10), bf16)
    w2rep[64 : 64 + _M2] = np.asarray(inputs["w2"], np.float32)[256:300].astype(bf16)
    b1rep = np.zeros((128, 1), np.float32)
    b1rep[64 : 64 + _M2, 0] = np.asarray(inputs["b1"], np.float32)[256:300]

    if "nc" not in _state:
        _state["nc"] = _build_nc()
    nc = _state["nc"]

    xb = x.astype(bf16)  # [65536, 784]
    in_maps = []
    for c in range(_NCORES):
        sh = np.zeros((_KP, _BSH), bf16)
        sh[:784] = xb[c * _BSH : (c + 1) * _BSH, :].T  # [784, 8192]
        for g in (1, 2, 3):
            sh[768 + 32 * g : 784 + 32 * g] = sh[768:784]
        in_maps.append(
            {
                "xt": sh,
                "w1": w1f,
                "b1": b1c,
                "w2": w2b,
                "b2f": b2f,
                "w2rep": w2rep,
                "b1rep": b1rep,
            }
        )

    res = run_bass_kernel_spmd(
        nc, in_maps, list(range(_NCORES)), trace=trace, **run_kwargs
    )
    outs = [res.results[c]["outT"] for c in range(_NCORES)]  # each [10, 8192]
    out = np.concatenate(outs, axis=1).T  # [65536, 10]
    return np.ascontiguousarray(out, dtype=np.float32), res


def kernel(**inputs):
    out, _ = _exec(inputs, trace=False)
    return out


# revision 20
# speedup vs baseline: 1.0624x; 1.0624x over previous
"""Trainium2 Bass kernel for nn_DigitConvolutionalModel.

Model: out = relu(conv2d_valid(x.reshape(28,28), conv_w).reshape(676) @ w1 + b1) @ w2 + b2

Strategy:
  - The 3x3 valid conv is a linear map C [784, 676]; fold it into the first
    FC layer on the host: W1' = C @ w1  [784, 300]. The device then runs a
    plain 2-layer MLP: out = relu(x @ W1' + b1) @ w2 + b2.
  - Pure data parallel over 8 NeuronCores: batch 65536 -> 8192 per core.
  - Feature-major device layout: host supplies x.T per core so the
    contraction dim sits on SBUF partitions for both matmul operands.
    Layer 1 computes g = relu(W1'.T @ x.T + b1) as [300, batch]; layer 2
    reuses g directly as the moving operand: out.T = w2.T @ g + b2.
  - All feature dims zero-padded to multiples of 128 (784->896, 300->384);
    the zero-padding contributes exactly 0 through matmul/relu.
  - bf16 matmul inputs (1 PE cycle/row vs 4 for fp32), fp32 PSUM accumulate.
  - Dependency tracking is tile-granular: a matmul reading one k-chunk slice
    of a tile waits for ALL writers of that tile. Pair-0 x and w1 therefore
    live in per-chunk tiles with per-chunk DMAs (k0 issued first) so the
    first real matmul starts as soon as chunk 0 lands (~8us) instead of
    after the whole prologue transfer (~13us).
  - m2 (44-row tail of the 300 hidden features) and layer 2 (M=10) both run
    col-tiled: tile_position col groups 0/64 give two concurrent streams on
    disjoint PE column groups, so each ki/mi slot costs one 512-col stream
    for both batch halves.
  - Per-pair block order [m01 | m2 | l2(prev)] keeps full-array/col-tiled
    mode switches to 3 per pair (each costs ~105ns of unhidden LDWEIGHTS).
  - Layer-2 output drains as ONE ACT pass over the whole [128,512] ps2 bank
    (j0 rows 0-9, j1 rows 64-73, bias from a [128,1] b2 replica); the two
    outT stores issue on gpsimd/vector so the final pair's stores overlap.
"""

import numpy as np
import ml_dtypes

_B = 65536
_NCORES = 8
_BSH = _B // _NCORES  # 8192 batch rows per core
_N = 512  # batch columns per matmul (one fp32 PSUM bank)
_KP = 896  # padded input features (784 -> 7 chunks of 128)
_MP = 384  # padded hidden features (300 -> 3 chunks of 128)
_NK = _KP // 128  # 7
_NM = _MP // 128  # 3
_NPAIR = _BSH // (2 * _N)  # 8 pairs of 512-col batch tiles
_M2 = 300 - 256  # 44 real rows in the third m-chunk

_state = {}


def _build_nc():
    import concourse.tile as tile
    from concourse import bacc, mybir
    from contextlib import ExitStack

    dt = mybir.dt
    AF = mybir.ActivationFunctionType

    nc = bacc.Bacc(
        "TRN2",
        target_bir_lowering=False,
        debug=False,
        enable_asserts=False,
        num_devices=_NCORES,
    )

    xt = nc.dram_tensor("xt", [_KP, _BSH], dt.bfloat16, kind="ExternalInput").ap()
    w1 = nc.dram_tensor("w1", [_KP, _MP], dt.bfloat16, kind="ExternalInput").ap()
    b1 = nc.dram_tensor("b1", [_MP, 1], dt.float32, kind="ExternalInput").ap()
    w2 = nc.dram_tensor("w2", [_MP, 10], dt.bfloat16, kind="ExternalInput").ap()
    b2f = nc.dram_tensor("b2f", [128, 1], dt.float32, kind="ExternalInput").ap()
    w2rep = nc.dram_tensor("w2rep", [128, 10], dt.bfloat16, kind="ExternalInput").ap()
    b1rep = nc.dram_tensor("b1rep", [128, 1], dt.float32, kind="ExternalInput").ap()
    outT = nc.dram_tensor("outT", [10, _BSH], dt.float32, kind="ExternalOutput").ap()

    # Partition-chunked DRAM views: [(chunk, p), cols] -> [p, chunk, cols]
    xt_r = xt.rearrange("(k p) c -> p k c", p=128)  # [128, 7, 8192]
    w1_r = w1.rearrange("(k p) m -> p k m", p=128)  # [128, 7, 384]
    b1_r = b1.rearrange("(m p) one -> p m one", p=128)  # [128, 3, 1]
    w2_r = w2.rearrange("(m p) o -> p m o", p=128)  # [128, 3, 10]

    with tile.TileContext(nc) as tc, ExitStack() as ctx:
        wpool = ctx.enter_context(tc.tile_pool(name="wpool", bufs=1))
        xp0 = ctx.enter_context(tc.tile_pool(name="xp0", bufs=1))
        xpool = ctx.enter_context(tc.tile_pool(name="xpool", bufs=2))
        gpool = ctx.enter_context(tc.tile_pool(name="gpool", bufs=2))
        ppool = ctx.enter_context(tc.tile_pool(name="ppool", bufs=5, space="PSUM"))
        pm2pool = ctx.enter_context(tc.tile_pool(name="pm2pool", bufs=1, space="PSUM"))
        p2pool = ctx.enter_context(tc.tile_pool(name="p2pool", bufs=2, space="PSUM"))
        opool = ctx.enter_context(tc.tile_pool(name="opool", bufs=2))

        # PE warm-up: a few dependency-free matmuls on a zeroed scratch tile
        # bridge the gap between engine start (~7.4us) and the first x/w
        # chunks landing (~8.3us) so the HAM activity window starts filling
        # and the PE never idles before the real stream begins. The memset
        # runs on vector so gpsimd's queue starts with the x-chunk DMAs.
        warm_in = wpool.tile([128, 128], dt.bfloat16, name="warm_in", tag="warm_in")
        nc.vector.memset(warm_in[:], 0.0)
        warm_ps = p2pool.tile([128, 128], dt.float32, name="warm_ps", tag="ps2")
        for _ in range(8):
            nc.tensor.matmul(
                out=warm_ps[:], lhsT=warm_in[:], rhs=warm_in[:], start=True, stop=True
            )

        # Pair-0 x and the stationary weights in PER-CHUNK tiles so the first
        # matmul only waits on its own (w1[k0], xt[k0]) chunk DMAs. Chunk
        # DMAs are ordered k0-first on two otherwise-idle engine queues
        # (weights on sync, x on gpsimd); each dma_start costs ~0.65us of
        # descriptor generation on the issuing engine.
        # x0 chunks go on scalar and w1 chunks on sync: both are HARDWARE
        # DGE rings that start transfers promptly. The gpsimd ring is a
        # software DGE whose transfers lag ~2-3us, so it only gets the
        # late-needed small tensors and the pair-1 prefetch.
        w1c = []
        xt0c = []
        for ki in range(_NK):
            wck = wpool.tile([128, _MP], dt.bfloat16, name=f"w1c_{ki}", tag=f"w1c{ki}")
            nc.sync.dma_start(out=wck[:], in_=w1_r[:, ki, :])
            w1c.append(wck)
            xck = xp0.tile([128, 2 * _N], dt.bfloat16, name=f"xt0_{ki}", tag=f"x0{ki}")
            nc.scalar.dma_start(out=xck[:], in_=xt_r[:, ki, 0 : 2 * _N])
            xt0c.append(xck)
        # Small tensors ride the sync HW ring after the w chunks; b1 first
        # (needed earliest, by pair-0's relu drain).
        b1sb = wpool.tile([128, _NM, 1], dt.float32, name="b1sb", tag="b1sb")
        nc.sync.dma_start(out=b1sb[:], in_=b1_r[:])
        w2sb = wpool.tile([128, _NM, 10], dt.bfloat16, name="w2sb", tag="w2sb")
        nc.sync.dma_start(out=w2sb[:], in_=w2_r[:])
        # b2 replicated at partitions 0-9 (j0) and 64-73 (j1) so one ACT pass
        # with per-partition bias drains both halves of the layer-2 psum.
        b2sb = wpool.tile([128, 1], dt.float32, name="b2sb", tag="b2sb")
        nc.sync.dma_start(out=b2sb[:], in_=b2f[:, :])
        # The m2 chunk (44 real rows of 300) is col-tiled: batch half j=1
        # lands at psum/sbuf partitions 64.. so its bias and layer-2 weights
        # need partition-64-aligned replicas (built fully on host: one DMA).
        b1rsb = wpool.tile([128, 1], dt.float32, name="b1rsb", tag="b1rsb")
        nc.sync.dma_start(out=b1rsb[:], in_=b1rep[:, :])
        w2rsb = wpool.tile([128, 10], dt.bfloat16, name="w2rsb", tag="w2rsb")
        nc.sync.dma_start(out=w2rsb[:], in_=w2rep[:, :])

        _KA = 4  # k-chunks in the first half-tile of a prefetched pair

        def xrhs(pair, xtile, ki, cols, rows=slice(0, 128)):
            if pair == 0:
                return xt0c[ki][rows, cols]
            xa, xb = xtile
            if ki < _KA:
                return xa[rows, ki, cols]
            return xb[rows, ki - _KA, cols]

        def layer2(prev_g, prev_c0):
            """Second layer for the pair at column prev_c0: both batch halves
            run concurrently on PE col groups 0 / 64 into one psum bank."""
            ps2 = p2pool.tile([128, _N], dt.float32, name=f"ps2_{prev_c0}", tag="ps2")
            for mi in range(_NM):
                for j in range(2):
                    if j == 0:
                        lw = w2sb[:, mi, :]
                    else:
                        # j1's m2 g-rows live at partitions 64-107; use the
                        # partition-aligned replica for that chunk.
                        lw = w2rsb[:] if mi == 2 else w2sb[:, mi, :]
                    nc.tensor.matmul(
                        out=ps2[64 * j : 64 * j + 10, :],
                        lhsT=lw,
                        rhs=prev_g[(mi, j)][:],
                        start=(mi == 0),
                        stop=(mi == _NM - 1),
                        tile_position=(0, 64 * j),
                    )
            # One ACT pass drains both halves (rows 0-9 and 64-73); the
            # untouched rows are never stored.
            ob = opool.tile([128, _N], dt.float32, name=f"ob_{prev_c0}", tag="ob")
            nc.scalar.activation(ob[:], ps2[:], AF.Identity, bias=b2sb[:], scale=1.0)
            nc.sync.dma_start(out=outT[:, prev_c0 : prev_c0 + _N], in_=ob[0:10, :])
            nc.scalar.dma_start(
                out=outT[:, prev_c0 + _N : prev_c0 + 2 * _N], in_=ob[64:74, :]
            )

        prev_g = None
        prev_c0 = 0
        for pair in range(_NPAIR):
            c0 = pair * 2 * _N
            xtile = None
            if pair > 0:
                # Prefetched pairs arrive as two half-tiles (k0-3, k4-6) so
                # the pair's first matmuls only wait on the first ~1MB and
                # the second half's DMA deadline is ~3.5us later. The start
                # is HBM-bandwidth-bound and rings share HBM fairly, so ALL
                # x transfers ride ONE ring (scalar) whose FIFO enforces the
                # consumption order: x0 chunks, then pair 1, 2, ... Each
                # pair's issue also self-paces behind the previous pair's
                # relu work on the scalar engine queue.
                xa = xpool.tile(
                    [128, _KA, 2 * _N], dt.bfloat16, name=f"xta_{pair}", tag="xa"
                )
                xb = xpool.tile(
                    [128, _NK - _KA, 2 * _N], dt.bfloat16, name=f"xtb_{pair}", tag="xb"
                )
                nc.scalar.dma_start(out=xa[:], in_=xt_r[:, 0:_KA, c0 : c0 + 2 * _N])
                nc.scalar.dma_start(out=xb[:], in_=xt_r[:, _KA:_NK, c0 : c0 + 2 * _N])
                xtile = (xa, xb)

            cur_g = {}
            # k-major sweep over m0/m1: consume each x-chunk for all four
            # (mi, j) accumulators before needing the next chunk, so the
            # DMA-raced first pair doesn't stall the TensorEngine.
            ps = {
                (mi, j): ppool.tile(
                    [128, _N], dt.float32, name=f"ps_{pair}_{mi}_{j}", tag="ps"
                )
                for mi in range(2)
                for j in range(2)
            }
            for ki in range(_NK - 1):
                for mi in range(2):
                    for j in range(2):
                        nc.tensor.matmul(
                            out=ps[(mi, j)][:],
                            lhsT=w1c[ki][:, mi * 128 : (mi + 1) * 128],
                            rhs=xrhs(pair, xtile, ki, slice(j * _N, (j + 1) * _N)),
                            start=(ki == 0),
                            stop=False,
                        )
            # ki=6 has only 16 real contraction rows (784 = 6*128 + 16). The
            # host replicates those rows (x and w1 alike) at partition
            # offsets 32/64/96 of chunk 6, so the four (mi, j) tails run as
            # K=16 row-tiled matmuls on distinct 32-row groups — which the
            # PE executes CONCURRENTLY (one ~512-cycle slot instead of 4).
            for gi, (mi, j) in enumerate([(0, 0), (0, 1), (1, 0), (1, 1)]):
                rg = 32 * gi
                nc.tensor.matmul(
                    out=ps[(mi, j)][:],
                    lhsT=w1c[_NK - 1][rg : rg + 16, mi * 128 : (mi + 1) * 128],
                    rhs=xrhs(
                        pair, xtile, _NK - 1,
                        slice(j * _N, (j + 1) * _N), rows=slice(rg, rg + 16),
                    ),
                    start=False,
                    stop=True,
                    tile_position=(rg, 0),
                )
            for mi in range(2):
                for j in range(2):
                    g = gpool.tile(
                        [128, _N], dt.bfloat16, name=f"g_{pair}_{mi}_{j}", tag=f"g{mi}{j}"
                    )
                    if j == 0:
                        # Split the relus across ACT and DVE so neither engine
                        # serializes the psum drain.
                        nc.scalar.activation(
                            g[:], ps[(mi, j)][:], AF.Relu, bias=b1sb[:, mi, :], scale=1.0
                        )
                    else:
                        nc.vector.tensor_scalar(
                            g[:], ps[(mi, j)][:], b1sb[:, mi, :], 0.0,
                            mybir.AluOpType.add, mybir.AluOpType.max,
                        )
                    cur_g[(mi, j)] = g

            # m2 chunk (44 output rows): both batch halves run concurrently as
            # col-tiled matmuls — j=0 writes psum partitions 0..43 (col group
            # 0), j=1 writes partitions 64..107 (col group 64) of one bank.
            psm2 = pm2pool.tile([128, _N], dt.float32, name=f"psm2_{pair}", tag="psm2")
            for ki in range(_NK):
                # Chunk 6 carries the 16 real rows replicated at partition
                # offsets 32/64/96 (for the m01 row-tiled tails); m2 must
                # contract over partitions 0-15 only or it would count the
                # replicas four times.
                kr = slice(0, 16) if ki == _NK - 1 else slice(0, 128)
                for j in range(2):
                    nc.tensor.matmul(
                        out=psm2[64 * j : 64 * j + _M2, :],
                        lhsT=w1c[ki][kr, 256 : 256 + _M2],
                        rhs=xrhs(pair, xtile, ki, slice(j * _N, (j + 1) * _N), rows=kr),
                        start=(ki == 0),
                        stop=(ki == _NK - 1),
                        tile_position=(0, 64 * j),
                    )
            # g tiles are full 128 rows with the unused rows zeroed so layer 2
            # can use uniform full-row matmuls (0-weight x 0-value, never NaN).
            # The gpool ring has 2 buffers per tag and nothing but these
            # memsets ever writes the padding rows, so zeroing the first two
            # generations keeps every later generation zero too.
            g20 = gpool.tile([128, _N], dt.bfloat16, name=f"g_{pair}_2_0", tag="g20")
            if pair < 2:
                nc.vector.memset(g20[32:64, :], 0.0)  # 32-aligned; relu rewrites 32..43
                nc.vector.memset(g20[64:128, :], 0.0)
            nc.scalar.activation(
                g20[0:_M2, :], psm2[0:_M2, :], AF.Relu, bias=b1sb[0:_M2, 2, :], scale=1.0
            )
            g21 = gpool.tile([128, _N], dt.bfloat16, name=f"g_{pair}_2_1", tag="g21")
            if pair < 2:
                nc.vector.memset(g21[0:64, :], 0.0)
                nc.vector.memset(g21[96:128, :], 0.0)  # 32-aligned; relu rewrites 96..107
            nc.vector.tensor_scalar(
                g21[64 : 64 + _M2, :], psm2[64 : 64 + _M2, :], b1rsb[64 : 64 + _M2, :],
                0.0, mybir.AluOpType.add, mybir.AluOpType.max,
            )
            cur_g[(2, 0)] = g20
            cur_g[(2, 1)] = g21

            # Software-pipelined layer 2 for the previous pair, placed after
            # m2 so the col-tiled blocks are adjacent (fewer PE mode switches).
            if prev_g is not None:
                layer2(prev_g, prev_c0)
            prev_g = cur_g
            prev_c0 = c0
        layer2(prev_g, prev_c0)

    nc.compile()
    return nc


def _fold_conv(conv_w, w1):
    """W1' = C @ w1 where C [784, 676] is the linear map of the 3x3 valid conv."""
    C = np.zeros((784, 676), np.float64)
    cw = np.asarray(conv_w, np.float64)
    for di in range(3):
        for dj in range(3):
            for i in range(26):
                rows = (i + di) * 28 + dj + np.arange(26)
                C[rows, i * 26 + np.arange(26)] += cw[di, dj]
    return C @ np.asarray(w1, np.float64)  # [784, 300]


def _exec(inputs, trace=False, **run_kwargs):
    from concourse.bass_utils import run_bass_kernel_spmd

    x = np.asarray(inputs["x"], np.float32)
    bf16 = ml_dtypes.bfloat16

    w1f = np.zeros((_KP, _MP), bf16)
    w1f[:784, :300] = _fold_conv(inputs["conv_w"], inputs["w1"]).astype(bf16)
    # Replicate the 16 real rows of k-chunk 6 at partition offsets 32/64/96
    # for the row-tiled ki=6 tail matmuls (see _build_nc).
    for g in (1, 2, 3):
        w1f[768 + 32 * g : 784 + 32 * g] = w1f[768:784]
    b1c = np.zeros((_MP, 1), np.float32)
    b1c[:300, 0] = np.asarray(inputs["b1"], np.float32)
    w2b = np.zeros((_MP, 10), bf16)
    w2b[:300] = np.asarray(inputs["w2"], np.float32).astype(bf16)
    b2v = np.asarray(inputs["b2"], np.float32).reshape(10)
    b2f = np.zeros((128, 1), np.float32)
    b2f[0:10, 0] = b2v
    b2f[64:74, 0] = b2v
    # Partition-64-aligned replicas for the m2 chunk's j=1 half.
    w2rep = np.zeros((128, 10), bf16)
    w2rep[64 : 64 + _M2] = np.asarray(inputs["w2"], np.float32)[256:300].astype(bf16)
    b1rep = np.zeros((128, 1), np.float32)
    b1rep[64 : 64 + _M2, 0] = np.asarray(inputs["b1"], np.float32)[256:300]

    if "nc" not in _state:
        _state["nc"] = _build_nc()
    nc = _state["nc"]

    xb = x.astype(bf16)  # [65536, 784]
    in_maps = []
    for c in range(_NCORES):
        sh = np.zeros((_KP, _BSH), bf16)
        sh[:784] = xb[c * _BSH : (c + 1) * _BSH, :].T  # [784, 8192]
        for g in (1, 2, 3):
            sh[768 + 32 * g : 784 + 32 * g] = sh[768:784]
        in_maps.append(
            {
                "xt": sh,
                "w1": w1f,
                "b1": b1c,
                "w2": w2b,
                "b2f": b2f,
                "w2rep": w2rep,
                "b1rep": b1rep,
            }
        )

    res = run_bass_kernel_spmd(
        nc, in_maps, list(range(_NCORES)), trace=trace, **run_kwargs
    )
    outs = [res.results[c]["outT"] for c in range(_NCORES)]  # each [10, 8192]
    out = np.concatenate(outs, axis=1).T  # [65536, 10]
    return np.ascontiguousarray(out, dtype=np.float32), res


def kernel(**inputs):
    out, _ = _exec(inputs, trace=False)
    return out
